# revision 41
# baseline (speedup 1.0000x reference)
"""ALiBi causal multihead attention on 8 TRN2 NeuronCores.

Sharding: (batch, head-half). Core c handles batch c//2 and the 8 heads
{2j + c%2} (interleaved so the per-slot ALiBi-sparsity skip pattern is
program-uniform across cores while each core still covers a spread of
slopes). Each core loads only its batch's activations (6.3 MB vs 25 MB
for head-only sharding), computes column-parallel Q/K/V projections for
its 512 dims, full attention for its 8 heads, and a row-parallel partial
output projection; the host sums the two partials per batch and adds bo.

ALiBi bias + causal mask: exp(score + bias) = exp(score) * EB where
EB[k, q] = exp(slope * (k - q)) * [k <= q] depends only on (k - q) —
Toeplitz. Each (head, 256-col-chunk) attention block multiplies by a
view of one of a handful of canonical [128, 256] EB tiles (off' = k0 -
q0 - c0), so no per-(h,qb,kc) bias tensors are loaded: ~45 tiles
(~3 MB) replace the 16.8 MB per-core bias tensor head-sharding needs.

Sparsity: scores farther than ~26/slope below the diagonal carry
weights < 1e-9 relative; per (slot, qb, kc) the live column span is
precomputed (128-aligned) and QK / exp / EB-mult / PV all trim to it.
The span table uses the shallower slope of each slot's two possible
heads, so the program is identical on every core.

Key padding: host zeroes masked key columns of x_v and ships a 0/1
column that lands in the V-augmentation "ones" slots, so masked keys
drop out of both the numerator and the softmax denominator exactly.

PV runs with pt as the stationary operand (moving = V-aug, 65 cols),
producing O in [q, d] orientation with the denominator on the same
partition as its queries: normalization is a per-partition
reciprocal_approx_fast + tensor_scalar_mul — no cross-partition
broadcast. PE [128,128] transposes then build O^T for the Wo matmuls.
"""

import math

import numpy as np
import ml_dtypes

B, S, D, H = 4, 1024, 1024, 16
DK = D // H  # 64
N_CORES = 8
NSLOT = 8  # heads per core
THETA = 26.0  # exp(-THETA) ~ 5e-12: ALiBi sparsity cutoff

BF16 = ml_dtypes.bfloat16

_BUILT = {}
_WAITSPLIT_N = [0]


def _slope(h):  # global head h (0-indexed), matches reference _alibi_bias
    return 2.0 ** (-8.0 * (h + 1) / H)


def _plan():
    """Program-uniform span table.

    spans[(j, qb, kc)] = (lo, hi): live query columns (within the 512-wide
    qb block, 128-aligned) of key chunk kc for head-slot j. Governing
    slope per slot is the shallower of its two possible heads (2j+1).
    Also returns, per (j, qb, qc), the first/last kc whose span covers
    query chunk qc (for PV psum start/stop flags), and the canonical EB
    tile offsets needed per slot.
    """
    spans = {}
    for j in range(NSLOT):
        sl = _slope(2 * j + 1)
        dmax = math.ceil(THETA / sl)
        for qb in range(2):
            q0 = qb * 512
            for kc in range(4 if qb == 0 else 8):
                k0 = kc * 128
                lo = max(0, k0 - q0)
                hi = min(512, k0 + 128 + dmax - q0)
                hi = min(512, ((hi + 127) // 128) * 128)
                if hi > lo:
                    spans[(j, qb, kc)] = (lo, hi)

    cover = {}  # (j, qb, qc) -> [kc, ...]
    for (j, qb, kc), (lo, hi) in spans.items():
        for qc in range(lo // 128, hi // 128):
            cover.setdefault((j, qb, qc), []).append(kc)
    for v in cover.values():
        v.sort()

    eboffs = {}  # j -> sorted list of off' values
    for (j, qb, kc), (lo, hi) in spans.items():
        off = kc * 128 - qb * 512
        for c0 in (0, 256):
            if max(lo, c0) < min(hi, c0 + 256):
                eboffs.setdefault(j, set()).add(off - c0)
    eboffs = {j: sorted(s) for j, s in eboffs.items()}
    ebbase = {}  # (j, off') -> tile index in the packed EB buffer
    n = 0
    for j in range(NSLOT):
        for o in eboffs[j]:
            ebbase[(j, o)] = n
            n += 1
    return spans, cover, eboffs, ebbase, n


SPANS, COVER, EBOFFS, EBBASE, NEB = _plan()


def _split_sync_waits(nc, limit=1):
    """This walrus build rejects instructions carrying more than ~1 sync
    wait. Strip excess waits onto dedicated same-engine nops spliced
    immediately before the instruction (same sequencer => same semantics)."""
    import concourse.mybir as mybir

    for fn in nc.m.functions:
        for bb in fn.blocks:
            out = []
            changed = False
            for inst in bb.instructions:
                si = inst.sync_info
                if si is not None and si.on_wait and len(si.on_wait) > limit:
                    waits = list(si.on_wait)
                    si.on_wait = waits[:limit]
                    for w in waits[limit:]:
                        _WAITSPLIT_N[0] += 1
                        nop = mybir.InstNoOp(
                            name=f"waitsplit_{_WAITSPLIT_N[0]}",
                            engine=inst.engine,
                            ins=[],
                            outs=[],
                            sync_info=mybir.SyncInfo(on_wait=[w], on_update=[]),
                        )
                        out.append(nop)
                    changed = True
                out.append(inst)
            if changed:
                bb.instructions = out


def _build(with_bias, split=True):
    import concourse.bass as bass
    import concourse.mybir as mybir
    from concourse import masks
    from concourse.tile import TileContext

    f32 = mybir.dt.float32
    bf16 = mybir.dt.bfloat16
    Exp = mybir.ActivationFunctionType.Exp

    nc = bass.Bass()

    xq = nc.declare_dram_parameter("xq", [D, S], bf16, isOutput=False)
    xk = nc.declare_dram_parameter("xk", [D, S], bf16, isOutput=False)
    xv = nc.declare_dram_parameter("xv", [D, S], bf16, isOutput=False)
    wq = nc.declare_dram_parameter("wq", [128, 4096], bf16, isOutput=False)
    wk = nc.declare_dram_parameter("wk", [128, 4096], bf16, isOutput=False)
    wv = nc.declare_dram_parameter("wv", [128, 4096], bf16, isOutput=False)
    wo = nc.declare_dram_parameter("wo", [128, 4096], bf16, isOutput=False)
    ebp = nc.declare_dram_parameter("ebp", [128, NEB * 256], bf16, isOutput=False)
    mcol = nc.declare_dram_parameter("mcol", [128, 64], bf16, isOutput=False)
    if with_bias:
        bqp = nc.declare_dram_parameter("bq", [128, 4], f32, isOutput=False)
        bkp = nc.declare_dram_parameter("bk", [128, 4], f32, isOutput=False)
        bvp = nc.declare_dram_parameter("bv", [1, 512], f32, isOutput=False)
    out = nc.declare_dram_parameter("out", [S, D], bf16, isOutput=True)

    with TileContext(nc) as tc:
        with (
            tc.tile_pool(name="const", bufs=1) as cpool,
            tc.tile_pool(name="xt", bufs=48) as xpool,
            tc.tile_pool(name="qk", bufs=1) as qkpool,
            tc.tile_pool(name="vs", bufs=1) as vpool,
            tc.tile_pool(name="et", bufs=6) as etp,
            tc.tile_pool(name="pt", bufs=18) as ptp,
            tc.tile_pool(name="oqd", bufs=8) as oqdp,
            tc.tile_pool(name="o8t", bufs=8) as o8p,
            tc.tile_pool(name="rc", bufs=4) as rcp,
            tc.tile_pool(name="ob", bufs=7) as obp,
            tc.tile_pool(name="psS", bufs=4, space="PSUM") as psS,
            tc.tile_pool(name="psO", bufs=2, space="PSUM") as psO,
            tc.tile_pool(name="psW", bufs=2, space="PSUM") as psW,
        ):
            # ---- constants / weights ----
            wq_sb = cpool.tile([128, 4096], bf16, tag="wq")
            wk_sb = cpool.tile([128, 4096], bf16, tag="wk")
            wv_sb = cpool.tile([128, 4096], bf16, tag="wv")
            wo_sb = cpool.tile([128, 4096], bf16, tag="wo")
            eb_sb = cpool.tile([128, NEB * 256], bf16, tag="eb")
            mc_sb = cpool.tile([128, 64], bf16, tag="mc")
            ident = cpool.tile([128, 128], bf16, tag="ident")
            if with_bias:
                bq_sb = cpool.tile([128, 4], f32, tag="bq")
                bk_sb = cpool.tile([128, 4], f32, tag="bk")
                bv_sb = cpool.tile([1, 512], f32, tag="bv")
                ones_sb = cpool.tile([1, 128], f32, tag="ones")
                bvbc_sb = cpool.tile([128, 512], f32, tag="bvbc")

            KT = [
                qkpool.tile([128, S], bf16, tag=f"kt{dc}", name=f"KT{dc}")
                for dc in range(4)
            ]
            QT = [
                qkpool.tile([128, S], bf16, tag=f"qt{dc}", name=f"QT{dc}")
                for dc in range(4)
            ]
            # V-aug: [key-in-chunk, kc * (slot * 65)]; col 64 of each group
            # holds the key-padding indicator (1 = live).
            vsb = vpool.tile([128, 8 * 520], bf16, tag="vsb")

            xts = {}

            def dma_x(nm, x, h):
                for icc in range(8):
                    t = xpool.tile(
                        [128, 512], bf16, tag="x", name=f"x{nm}{icc}{h}"
                    )
                    nc.sync.dma_start(
                        out=t[:],
                        in_=x[icc * 128 : (icc + 1) * 128,
                              h * 512 : (h + 1) * 512],
                    )
                    xts[(nm, icc, h)] = t

            nc.sync.dma_start(out=wk_sb[:], in_=wk[:])
            nc.sync.dma_start(out=wq_sb[:], in_=wq[:])
            if with_bias:
                nc.sync.dma_start(out=bk_sb[:], in_=bkp[:])
                nc.sync.dma_start(out=bq_sb[:], in_=bqp[:])
                nc.sync.dma_start(out=bv_sb[:], in_=bvp[:])
            dma_x("k", xk, 0)
            dma_x("q", xq, 0)
            nc.sync.dma_start(out=wv_sb[:], in_=wv[:])
            nc.sync.dma_start(out=mc_sb[:], in_=mcol[:])
            nc.sync.dma_start(out=eb_sb[:], in_=ebp[:])
            dma_x("v", xv, 0)
            dma_x("k", xk, 1)
            dma_x("q", xq, 1)
            dma_x("v", xv, 1)
            nc.sync.dma_start(out=wo_sb[:], in_=wo[:])

            masks.make_identity(nc, ident[:])

            if with_bias:
                nc.vector.memset(ones_sb[:], 1.0)
                ps_bv = psW.tile([128, 512], f32, tag="pw", name="psbv")
                nc.tensor.matmul(
                    ps_bv[:], lhsT=ones_sb[:], rhs=bv_sb[:], start=True, stop=True
                )
                nc.vector.tensor_copy(bvbc_sb[:], ps_bv[:])

            def copy_to(eng, dst, src):
                if eng is nc.scalar:
                    nc.scalar.copy(dst, src)
                else:
                    eng.tensor_copy(dst, src)

            def proj_unit(nm, w_sb, dst, dc, half, b_sb, eng):
                """One [128 out-dims, 512 seq] projection block -> dst SBUF."""
                ps = psW.tile([128, 512], f32, tag="pw", name=f"pp{nm}{dc}{half}")
                for ic in range(8):
                    nc.tensor.matmul(
                        ps[:],
                        lhsT=w_sb[:, dc * 1024 + ic * 128 : dc * 1024 + (ic + 1) * 128],
                        rhs=xts[(nm, ic, half)][:],
                        start=(ic == 0),
                        stop=(ic == 7),
                    )
                if with_bias:
                    beng = eng if eng is not nc.scalar else nc.vector
                    beng.tensor_scalar_add(
                        out=dst, in0=ps[:], scalar1=b_sb[:, dc : dc + 1]
                    )
                else:
                    copy_to(eng, dst, ps[:])

            def v_unit(kc):
                """V projection for key chunk kc -> vsb strided (+pad col)."""
                ps = psW.tile([128, 512], f32, tag="pw", name=f"pv{kc}")
                for ic in range(8):
                    nc.tensor.matmul(
                        ps[:],
                        lhsT=xts[("v", ic, kc // 4)][
                            :, (kc % 4) * 128 : (kc % 4 + 1) * 128
                        ],
                        rhs=wv_sb[:, ic * 512 : (ic + 1) * 512],
                        start=(ic == 0),
                        stop=(ic == 7),
                    )
                dst = vsb[:, kc * 520 : (kc + 1) * 520].rearrange(
                    "p (j d) -> p j d", d=65
                )[:, :, 0:64]
                src = ps[:].rearrange("p (j d) -> p j d", d=64)
                if with_bias:
                    bcv = bvbc_sb[:].rearrange("p (j d) -> p j d", d=64)
                    nc.vector.tensor_add(dst, src, bcv)
                else:
                    nc.vector.tensor_copy(dst, src)
                nc.gpsimd.tensor_copy(
                    vsb[:, kc * 520 + 64 : (kc + 1) * 520 : 65],
                    mc_sb[:, kc * 8 : (kc + 1) * 8],
                )

            # ---- projections: only K(dc0) + Q-qb0(dc0) up front (slot 0/1
            # attention needs just those); the rest interleave with
            # attention steps so the PE never drains and ACT starts early.
            def k_unit(dc, half):
                proj_unit(
                    "k", wk_sb, KT[dc][:, half * 512 : (half + 1) * 512],
                    dc, half, bk_sb if with_bias else None, nc.scalar,
                )

            def q_unit(dc, half):
                proj_unit(
                    "q", wq_sb, QT[dc][:, half * 512 : (half + 1) * 512],
                    dc, half, bq_sb if with_bias else None,
                    nc.scalar if half == 0 else nc.vector,
                )

            for dc in range(4):
                k_unit(dc, 0)
            for dc in range(4):
                q_unit(dc, 0)
            pending_kq = [lambda dc=dc: k_unit(dc, 1) for dc in range(4)]
            pending_v = [lambda kc=kc: v_unit(kc) for kc in range(8)]
            pending_q1 = [lambda dc=dc: q_unit(dc, 1) for dc in range(4)]
            v_next = [0]

            def need_v(kc):
                while v_next[0] <= kc and pending_v:
                    pending_v.pop(0)()
                    v_next[0] += 1

            mulc = [0]
            copc = [0]
            oqd_tiles = {}

            def attn_front(j, qb, kc):
                lo, hi = SPANS[(j, qb, kc)]
                w = hi - lo
                dc, rb = j // 2, (j % 2) * 64
                ps = psS.tile([128, 512], f32, tag="ps", name=f"ps{j}_{qb}_{kc}")
                nc.tensor.matmul(
                    ps[:, 0:w],
                    lhsT=KT[dc][rb : rb + 64, kc * 128 : (kc + 1) * 128],
                    rhs=QT[dc][rb : rb + 64, qb * 512 + lo : qb * 512 + hi],
                    start=True,
                    stop=True,
                )
                et = etp.tile([128, 512], bf16, tag="et", name=f"et{j}_{qb}_{kc}")
                nc.scalar.activation(et[:, 0:w], ps[:, 0:w], Exp)
                pt = ptp.tile([128, 512], bf16, tag="pt", name=f"pt{j}_{qb}_{kc}")
                off = kc * 128 - qb * 512
                for c0 in (0, 256):
                    a, bnd = max(lo, c0), min(hi, c0 + 256)
                    if a >= bnd:
                        continue
                    base = EBBASE[(j, off - c0)] * 256
                    mulc[0] += 1
                    eng = nc.gpsimd if mulc[0] % 6 == 0 else nc.vector
                    eng.tensor_mul(
                        pt[:, a - lo : bnd - lo],
                        et[:, a - lo : bnd - lo],
                        eb_sb[:, base + a - c0 : base + bnd - c0],
                    )
                return pt

            def pv_cluster(j, qb, pts):
                """PV matmuls for one (slot, qb), grouped by query chunk so
                each psum accumulation group opens and closes before the
                next starts (hw corrupts an open group when another group
                starts in the same bank). Returns a list of closures."""
                po = psO.tile([128, 512], f32, tag="po", name=f"po{j}_{qb}")
                ops = []

                def pv(qc, kc):
                    need_v(kc)  # vsb writer must be emitted before this read
                    lo, hi = SPANS[(j, qb, kc)]
                    kcs = COVER[(j, qb, qc)]
                    nc.tensor.matmul(
                        po[:, qc * 65 : (qc + 1) * 65],
                        lhsT=pts[kc][:, qc * 128 - lo : qc * 128 - lo + 128],
                        rhs=vsb[:, kc * 520 + j * 65 : kc * 520 + (j + 1) * 65],
                        start=(kc == kcs[0]),
                        stop=(kc == kcs[-1]),
                        skip_group_check=True,
                    )

                for qc in range(4):
                    if (j, qb, qc) not in COVER:
                        continue
                    for kc in COVER[(j, qb, qc)]:
                        ops.append(lambda qc=qc, kc=kc: pv(qc, kc))
                ops.append(lambda: norm(j, qb, po))
                return ops

            def norm(j, qb, po):
                den = rcp.tile([128, 4], f32, tag="rc", name=f"dn{j}{qb}")
                nc.vector.tensor_copy(den[:], po[:, 64:260:65])
                rc = rcp.tile([128, 4], f32, tag="rc", name=f"rc{j}{qb}")
                nc.vector.reciprocal(rc[:], den[:])
                if qb not in oqd_tiles:
                    oqd_tiles[qb] = [
                        oqdp.tile([128, 512], bf16, tag="oqd", name=f"oq{qb}_{qc}")
                        for qc in range(4)
                    ]
                for qc in range(4):
                    nc.vector.tensor_scalar_mul(
                        out=oqd_tiles[qb][qc][:, j * 64 : (j + 1) * 64],
                        in0=po[:, qc * 65 : qc * 65 + 64],
                        scalar1=rc[:, qc : qc + 1],
                    )

            def transpose_dc(qb, qc, dc, s8):
                pst = psW.tile([128, 128], bf16, tag="pw", name=f"tr{s8}{dc}")
                nc.tensor.transpose(
                    pst[:], oqd_tiles[qb][qc][:, dc * 128 : (dc + 1) * 128],
                    ident[:],
                )
                o8 = o8p.tile([128, 128], bf16, tag="o8", name=f"o8{s8}{dc}")
                copc[0] += 1
                eng = nc.vector if copc[0] % 2 == 0 else nc.scalar
                copy_to(eng, o8[:], pst[:])
                return o8

            def wo_unit(qb, qc):
                s8 = qb * 4 + qc
                o8t = [transpose_dc(qb, qc, dc, s8) for dc in range(4)]
                ob = obp.tile([128, 1024], bf16, tag="ob", name=f"ob{s8}")
                for half in range(2):
                    pw = psW.tile([128, 512], f32, tag="pw", name=f"wo{s8}{half}")
                    for dc in range(4):
                        nc.tensor.matmul(
                            pw[:],
                            lhsT=o8t[dc][:],
                            rhs=wo_sb[:, dc * 1024 + half * 512 : dc * 1024 + (half + 1) * 512],
                            start=(dc == 0),
                            stop=(dc == 3),
                        )
                    eng = nc.vector if half == 0 else nc.scalar
                    copy_to(eng, ob[:, half * 512 : (half + 1) * 512], pw[:])
                nc.sync.dma_start(
                    out=out[s8 * 128 : (s8 + 1) * 128, :], in_=ob[:]
                )

            obA = {}

            def woA_unit(qc):
                # qb1 Wo, dims half A (head-slots 0-3): runs as soon as their
                # norms land so only half the contraction remains in the tail.
                s8 = 4 + qc
                o8t = [transpose_dc(1, qc, dc, s8) for dc in range(2)]
                oa = obp.tile([128, 1024], bf16, tag="ob", name=f"oa{s8}")
                obA[qc] = oa
                for half in range(2):
                    pw = psW.tile([128, 512], f32, tag="pw", name=f"woA{s8}{half}")
                    for dc in range(2):
                        nc.tensor.matmul(
                            pw[:],
                            lhsT=o8t[dc][:],
                            rhs=wo_sb[:, dc * 1024 + half * 512 : dc * 1024 + (half + 1) * 512],
                            start=(dc == 0),
                            stop=(dc == 1),
                        )
                    eng = nc.vector if half == 0 else nc.scalar
                    copy_to(eng, oa[:, half * 512 : (half + 1) * 512], pw[:])

            def woB_unit(qc):
                s8 = 4 + qc
                o8t = [transpose_dc(1, qc, dc, s8) for dc in (2, 3)]
                ob = obp.tile([128, 1024], bf16, tag="ob", name=f"ob{s8}")
                for half in range(2):
                    pw = psW.tile([128, 512], f32, tag="pw", name=f"woB{s8}{half}")
                    for dcx, dc in enumerate((2, 3)):
                        nc.tensor.matmul(
                            pw[:],
                            lhsT=o8t[dcx][:],
                            rhs=wo_sb[:, dc * 1024 + half * 512 : dc * 1024 + (half + 1) * 512],
                            start=(dcx == 0),
                            stop=(dcx == 1),
                        )
                    nc.vector.tensor_add(
                        ob[:, half * 512 : (half + 1) * 512],
                        pw[:],
                        obA[qc][:, half * 512 : (half + 1) * 512],
                    )
                nc.sync.dma_start(
                    out=out[s8 * 128 : (s8 + 1) * 128, :], in_=ob[:]
                )

            # ---- software-pipelined attention ----
            # Slot (j, qb) fronts (QK -> exp -> EB-mult) interleave with the
            # previous slot's deferred PV cluster + norm, plus V/Q-qb1
            # projection units and qb0 Wo units, to keep every engine fed.
            # qb0 steep-first (small early clusters while V is still
            # projecting); qb1 steep-last (smallest possible tail cluster).
            slot_list = [(0, j) for j in range(NSLOT)]
            slot_list += [(1, j) for j in range(NSLOT)]
            back_q = []
            wo_q = []
            gi = [0]
            def bg_pop():
                if pending_v:
                    pending_v.pop(0)()
                    v_next[0] += 1
                elif pending_kq:
                    pending_kq.pop(0)()
                elif pending_q1:
                    pending_q1.pop(0)()

            for si, (qb, j) in enumerate(slot_list):
                if qb == 1 and (pending_kq or pending_q1):
                    while pending_kq:
                        pending_kq.pop(0)()
                    while pending_q1:
                        pending_q1.pop(0)()
                if si == 9:
                    wo_q.extend(lambda qc=qc: wo_unit(0, qc) for qc in range(4))
                if si == 13:
                    wo_q.extend(lambda qc=qc: woA_unit(qc) for qc in range(4))
                kcs = [
                    kc
                    for kc in range(4 if qb == 0 else 8)
                    if (j, qb, kc) in SPANS
                ]
                pts = {}
                for t, kc in enumerate(kcs):
                    pts[kc] = attn_front(j, qb, kc)
                    gi[0] += 1
                    if pending_v or pending_kq or pending_q1:
                        if gi[0] % 3 == 1 or len(pending_v) > 4:
                            bg_pop()
                    rem = len(kcs) - t
                    k = -(-len(back_q) // rem)  # empty back_q by slot end
                    for _ in range(k):
                        back_q.pop(0)()
                    if not back_q and wo_q and gi[0] % 2 == 0:
                        wo_q.pop(0)()
                while back_q:
                    back_q.pop(0)()
                back_q = pv_cluster(j, qb, pts)
            while back_q:
                back_q.pop(0)()
            while pending_kq or pending_v or pending_q1:
                bg_pop()
            while wo_q:
                wo_q.pop(0)()
            for qc in range(4):
                woB_unit(qc)

    if split:
        _split_sync_waits(nc)
    return nc


def _get_built(with_bias):
    key = bool(with_bias)
    if key not in _BUILT:
        _BUILT[key] = _build(key)
    return _BUILT[key]


def _prepare(inputs):
    query = np.asarray(inputs["query"], np.float32)
    key = np.asarray(inputs["key"], np.float32)
    value = np.asarray(inputs["value"], np.float32)
    kpm = np.asarray(inputs["key_padding_mask"], bool)
    Wq = np.asarray(inputs["Wq"], np.float32)
    bq = np.asarray(inputs["bq"], np.float32)
    Wk = np.asarray(inputs["Wk"], np.float32)
    bk = np.asarray(inputs["bk"], np.float32)
    Wv = np.asarray(inputs["Wv"], np.float32)
    bv = np.asarray(inputs["bv"], np.float32)
    Wo = np.asarray(inputs["Wo"], np.float32)

    scale = 1.0 / np.sqrt(np.float32(DK))
    with_bias = bool(np.any(bq) or np.any(bk) or np.any(bv))

    xq_b = [
        np.ascontiguousarray(query[b].T).astype(BF16) for b in range(B)
    ]
    xk_b = [np.ascontiguousarray(key[b].T).astype(BF16) for b in range(B)]
    xv_b = []
    for b in range(B):
        v = value[b].T.copy()  # [D, S]
        v[:, kpm[b]] = 0.0
        xv_b.append(np.ascontiguousarray(v).astype(BF16))
    # mcol[p, kc*8 + j] = live[kc*128 + p] for every slot j
    mcol_b = []
    for b in range(B):
        live = (~kpm[b]).astype(np.float32).reshape(8, 128)  # [kc, p]
        m = np.repeat(live.T[:, :, None], 8, axis=2)  # [p, kc, j]
        mcol_b.append(np.ascontiguousarray(m.reshape(128, 64)).astype(BF16))

    def pack_w(Ws):  # [512, 1024] -> [128, 4096] lhsT tiles (dc, ic)
        return np.ascontiguousarray(
            Ws.reshape(4, 128, 8, 128).transpose(3, 0, 2, 1).reshape(128, 4096)
        )

    def pack_wv(Ws):  # [512, 1024] -> rhs tiles [128, ic*512]
        return np.ascontiguousarray(
            Ws.T.reshape(8, 128, 512).transpose(1, 0, 2).reshape(128, 4096)
        )

    in_maps = []
    for c in range(N_CORES):
        b, half = c // 2, c % 2
        heads = [2 * j + half for j in range(NSLOT)]
        dsel = np.concatenate([np.arange(h * DK, (h + 1) * DK) for h in heads])
        wq_c = (Wq[dsel, :] * scale).astype(BF16)
        wk_c = Wk[dsel, :].astype(BF16)
        wv_c = Wv[dsel, :].astype(BF16)
        wo_c = np.ascontiguousarray(Wo[:, dsel].T).astype(np.float32)  # [512,1024]
        # wo rhs tiles: [128 dims(dc), 4 dc * (1024 outs)]
        wo_pack = np.ascontiguousarray(
            wo_c.reshape(4, 128, 1024).transpose(1, 0, 2).reshape(128, 4096)
        ).astype(BF16)

        eb = np.zeros((128, NEB * 256), np.float32)
        pp = np.arange(128)[:, None]
        cc = np.arange(256)[None, :]
        for j in range(NSLOT):
            sl = _slope(heads[j])
            for o in EBOFFS[j]:
                t = (pp - cc + o).astype(np.float32)
                tile = np.where(t <= 0, np.exp(sl * np.minimum(t, 0.0)), 0.0)
                eb[:, EBBASE[(j, o)] * 256 : (EBBASE[(j, o)] + 1) * 256] = tile
        im = {
            "xq": xq_b[b],
            "xk": xk_b[b],
            "xv": xv_b[b],
            "wq": pack_w(wq_c.astype(np.float32)).astype(BF16),
            "wk": pack_w(wk_c.astype(np.float32)).astype(BF16),
            "wv": pack_wv(wv_c.astype(np.float32)).astype(BF16),
            "wo": wo_pack,
            "ebp": eb.astype(BF16),
            "mcol": mcol_b[b],
        }
        if with_bias:
            im["bq"] = (bq[dsel] * scale).astype(np.float32).reshape(4, 128).T.copy()
            im["bk"] = bk[dsel].astype(np.float32).reshape(4, 128).T.copy()
            im["bv"] = bv[dsel].astype(np.float32).reshape(1, 512)
        in_maps.append(im)
    return with_bias, in_maps


def _run(inputs, trace=False):
    from concourse.bass_utils import run_bass_kernel_spmd

    with_bias, in_maps = _prepare(inputs)
    nc = _get_built(with_bias)
    res = run_bass_kernel_spmd(nc, in_maps, list(range(N_CORES)), trace=trace)
    acc = np.zeros((B, S, D), np.float32)
    for c in range(N_CORES):
        acc[c // 2] += np.asarray(res.results[c]["out"], np.float32)
    acc += np.asarray(inputs["bo"], np.float32)[None, None, :]
    return acc, res


def kernel(**inputs):
    out, _ = _run(inputs)
    return out


# revision 42
# speedup vs baseline: 1.0502x; 1.0502x over previous
"""ALiBi causal multihead attention on 8 TRN2 NeuronCores.

Sharding: (batch, head-half). Core c handles batch c//2 and the 8 heads
{2j + c%2} (interleaved so the per-slot ALiBi-sparsity skip pattern is
program-uniform across cores while each core still covers a spread of
slopes). Each core loads only its batch's activations (6.3 MB vs 25 MB
for head-only sharding), computes column-parallel Q/K/V projections for
its 512 dims, full attention for its 8 heads, and a row-parallel partial
output projection; the host sums the two partials per batch and adds bo.

ALiBi bias + causal mask: exp(score + bias) = exp(score) * EB where
EB[k, q] = exp(slope * (k - q)) * [k <= q] depends only on (k - q) —
Toeplitz. Each (head, 256-col-chunk) attention block multiplies by a
view of one of a handful of canonical [128, 256] EB tiles (off' = k0 -
q0 - c0), so no per-(h,qb,kc) bias tensors are loaded: ~45 tiles
(~3 MB) replace the 16.8 MB per-core bias tensor head-sharding needs.

Sparsity: scores farther than ~26/slope below the diagonal carry
weights < 1e-9 relative; per (slot, qb, kc) the live column span is
precomputed (128-aligned) and QK / exp / EB-mult / PV all trim to it.
The span table uses the shallower slope of each slot's two possible
heads, so the program is identical on every core.

Key padding: host zeroes masked key columns of x_v and ships a 0/1
column that lands in the V-augmentation "ones" slots, so masked keys
drop out of both the numerator and the softmax denominator exactly.

PV runs with pt as the stationary operand (moving = V-aug, 65 cols),
producing O in [q, d] orientation with the denominator on the same
partition as its queries: normalization is a per-partition
reciprocal_approx_fast + tensor_scalar_mul — no cross-partition
broadcast. PE [128,128] transposes then build O^T for the Wo matmuls.
"""

import math

import numpy as np
import ml_dtypes

B, S, D, H = 4, 1024, 1024, 16
DK = D // H  # 64
N_CORES = 8
NSLOT = 8  # heads per core
THETA = 26.0  # exp(-THETA) ~ 5e-12: ALiBi sparsity cutoff

BF16 = ml_dtypes.bfloat16

_BUILT = {}
_WAITSPLIT_N = [0]


def _slope(h):  # global head h (0-indexed), matches reference _alibi_bias
    return 2.0 ** (-8.0 * (h + 1) / H)


def _plan():
    """Program-uniform span table.

    spans[(j, qb, kc)] = (lo, hi): live query columns (within the 512-wide
    qb block, 128-aligned) of key chunk kc for head-slot j. Governing
    slope per slot is the shallower of its two possible heads (2j+1).
    Also returns, per (j, qb, qc), the first/last kc whose span covers
    query chunk qc (for PV psum start/stop flags), and the canonical EB
    tile offsets needed per slot.
    """
    spans = {}
    for j in range(NSLOT):
        sl = _slope(2 * j + 1)
        dmax = math.ceil(THETA / sl)
        for qb in range(2):
            q0 = qb * 512
            for kc in range(4 if qb == 0 else 8):
                k0 = kc * 128
                lo = max(0, k0 - q0)
                hi = min(512, k0 + 128 + dmax - q0)
                hi = min(512, ((hi + 127) // 128) * 128)
                if hi > lo:
                    spans[(j, qb, kc)] = (lo, hi)

    cover = {}  # (j, qb, qc) -> [kc, ...]
    for (j, qb, kc), (lo, hi) in spans.items():
        for qc in range(lo // 128, hi // 128):
            cover.setdefault((j, qb, qc), []).append(kc)
    for v in cover.values():
        v.sort()

    eboffs = {}  # j -> sorted list of off' values
    for (j, qb, kc), (lo, hi) in spans.items():
        off = kc * 128 - qb * 512
        for c0 in (0, 256):
            if max(lo, c0) < min(hi, c0 + 256):
                eboffs.setdefault(j, set()).add(off - c0)
    eboffs = {j: sorted(s) for j, s in eboffs.items()}
    ebbase = {}  # (j, off') -> tile index in the packed EB buffer
    n = 0
    for j in range(NSLOT):
        for o in eboffs[j]:
            ebbase[(j, o)] = n
            n += 1
    return spans, cover, eboffs, ebbase, n


SPANS, COVER, EBOFFS, EBBASE, NEB = _plan()


def _split_sync_waits(nc, limit=1):
    """This walrus build rejects instructions carrying more than ~1 sync
    wait. Strip excess waits onto dedicated same-engine nops spliced
    immediately before the instruction (same sequencer => same semantics)."""
    import concourse.mybir as mybir

    for fn in nc.m.functions:
        for bb in fn.blocks:
            out = []
            changed = False
            for inst in bb.instructions:
                si = inst.sync_info
                if si is not None and si.on_wait and len(si.on_wait) > limit:
                    waits = list(si.on_wait)
                    si.on_wait = waits[:limit]
                    for w in waits[limit:]:
                        _WAITSPLIT_N[0] += 1
                        nop = mybir.InstNoOp(
                            name=f"waitsplit_{_WAITSPLIT_N[0]}",
                            engine=inst.engine,
                            ins=[],
                            outs=[],
                            sync_info=mybir.SyncInfo(on_wait=[w], on_update=[]),
                        )
                        out.append(nop)
                    changed = True
                out.append(inst)
            if changed:
                bb.instructions = out


def _build(with_bias, split=True):
    import concourse.bass as bass
    import concourse.mybir as mybir
    from concourse import masks
    from concourse.tile import TileContext

    f32 = mybir.dt.float32
    bf16 = mybir.dt.bfloat16
    Exp = mybir.ActivationFunctionType.Exp

    nc = bass.Bass()

    xq = nc.declare_dram_parameter("xq", [D, S], bf16, isOutput=False)
    xk = nc.declare_dram_parameter("xk", [D, S], bf16, isOutput=False)
    xv = nc.declare_dram_parameter("xv", [D, S], bf16, isOutput=False)
    wq = nc.declare_dram_parameter("wq", [128, 4096], bf16, isOutput=False)
    wk = nc.declare_dram_parameter("wk", [128, 4096], bf16, isOutput=False)
    wv = nc.declare_dram_parameter("wv", [128, 4096], bf16, isOutput=False)
    wo = nc.declare_dram_parameter("wo", [128, 4096], bf16, isOutput=False)
    ebp = nc.declare_dram_parameter("ebp", [128, NEB * 256], bf16, isOutput=False)
    mcol = nc.declare_dram_parameter("mcol", [128, 64], bf16, isOutput=False)
    if with_bias:
        bqp = nc.declare_dram_parameter("bq", [128, 4], f32, isOutput=False)
        bkp = nc.declare_dram_parameter("bk", [128, 4], f32, isOutput=False)
        bvp = nc.declare_dram_parameter("bv", [1, 512], f32, isOutput=False)
    out = nc.declare_dram_parameter("out", [S, D], bf16, isOutput=True)

    with TileContext(nc) as tc:
        with (
            tc.tile_pool(name="const", bufs=1) as cpool,
            tc.tile_pool(name="xt", bufs=24) as xpool,
            tc.tile_pool(name="qk", bufs=1) as qkpool,
            tc.tile_pool(name="vs", bufs=1) as vpool,
            tc.tile_pool(name="et", bufs=6) as etp,
            tc.tile_pool(name="pt", bufs=18) as ptp,
            tc.tile_pool(name="oqd", bufs=8) as oqdp,
            tc.tile_pool(name="o8t", bufs=8) as o8p,
            tc.tile_pool(name="rc", bufs=4) as rcp,
            tc.tile_pool(name="ob", bufs=7) as obp,
            tc.tile_pool(name="psS", bufs=4, space="PSUM") as psS,
            tc.tile_pool(name="psO", bufs=2, space="PSUM") as psO,
            tc.tile_pool(name="psW", bufs=2, space="PSUM") as psW,
        ):
            # ---- constants / weights ----
            wq_sb = cpool.tile([128, 4096], bf16, tag="wq")
            wk_sb = cpool.tile([128, 4096], bf16, tag="wk")
            wv_sb = cpool.tile([128, 4096], bf16, tag="wv")
            wo_sb = cpool.tile([128, 4096], bf16, tag="wo")
            eb_sb = cpool.tile([128, NEB * 256], bf16, tag="eb")
            mc_sb = cpool.tile([128, 64], bf16, tag="mc")
            ident = cpool.tile([128, 128], bf16, tag="ident")
            if with_bias:
                bq_sb = cpool.tile([128, 4], f32, tag="bq")
                bk_sb = cpool.tile([128, 4], f32, tag="bk")
                bv_sb = cpool.tile([1, 512], f32, tag="bv")
                ones_sb = cpool.tile([1, 128], f32, tag="ones")
                bvbc_sb = cpool.tile([128, 512], f32, tag="bvbc")

            KT = [
                qkpool.tile([128, S], bf16, tag=f"kt{dc}", name=f"KT{dc}")
                for dc in range(4)
            ]
            QT = [
                qkpool.tile([128, S], bf16, tag=f"qt{dc}", name=f"QT{dc}")
                for dc in range(4)
            ]
            # V-aug: [key-in-chunk, kc * (slot * 65)]; col 64 of each group
            # holds the key-padding indicator (1 = live).
            vsb = vpool.tile([128, 8 * 520], bf16, tag="vsb")

            xts = {}

            def dma_x(nm, x):
                for icc in range(8):
                    t = xpool.tile([128, S], bf16, tag="x", name=f"x{nm}{icc}")
                    nc.sync.dma_start(out=t[:], in_=x[icc * 128 : (icc + 1) * 128, :])
                    xts[(nm, icc)] = t

            nc.sync.dma_start(out=wk_sb[:], in_=wk[:])
            if with_bias:
                nc.sync.dma_start(out=bk_sb[:], in_=bkp[:])
                nc.sync.dma_start(out=bq_sb[:], in_=bqp[:])
                nc.sync.dma_start(out=bv_sb[:], in_=bvp[:])
            dma_x("k", xk)
            nc.sync.dma_start(out=wq_sb[:], in_=wq[:])
            dma_x("q", xq)
            nc.sync.dma_start(out=wv_sb[:], in_=wv[:])
            dma_x("v", xv)
            nc.sync.dma_start(out=mc_sb[:], in_=mcol[:])
            nc.sync.dma_start(out=eb_sb[:], in_=ebp[:])
            nc.sync.dma_start(out=wo_sb[:], in_=wo[:])

            masks.make_identity(nc, ident[:])

            if with_bias:
                nc.vector.memset(ones_sb[:], 1.0)
                ps_bv = psW.tile([128, 512], f32, tag="pw", name="psbv")
                nc.tensor.matmul(
                    ps_bv[:], lhsT=ones_sb[:], rhs=bv_sb[:], start=True, stop=True
                )
                nc.vector.tensor_copy(bvbc_sb[:], ps_bv[:])

            def copy_to(eng, dst, src):
                if eng is nc.scalar:
                    nc.scalar.copy(dst, src)
                else:
                    eng.tensor_copy(dst, src)

            def proj_unit(nm, w_sb, dst, dc, half, b_sb, eng):
                """One [128 out-dims, 512 seq] projection block -> dst SBUF."""
                ps = psW.tile([128, 512], f32, tag="pw", name=f"pp{nm}{dc}{half}")
                for ic in range(8):
                    nc.tensor.matmul(
                        ps[:],
                        lhsT=w_sb[:, dc * 1024 + ic * 128 : dc * 1024 + (ic + 1) * 128],
                        rhs=xts[(nm, ic)][:, half * 512 : (half + 1) * 512],
                        start=(ic == 0),
                        stop=(ic == 7),
                    )
                if with_bias:
                    beng = eng if eng is not nc.scalar else nc.vector
                    beng.tensor_scalar_add(
                        out=dst, in0=ps[:], scalar1=b_sb[:, dc : dc + 1]
                    )
                else:
                    copy_to(eng, dst, ps[:])

            def v_unit(kc):
                """V projection for key chunk kc -> vsb strided (+pad col)."""
                ps = psW.tile([128, 512], f32, tag="pw", name=f"pv{kc}")
                for ic in range(8):
                    nc.tensor.matmul(
                        ps[:],
                        lhsT=xts[("v", ic)][:, kc * 128 : (kc + 1) * 128],
                        rhs=wv_sb[:, ic * 512 : (ic + 1) * 512],
                        start=(ic == 0),
                        stop=(ic == 7),
                    )
                dst = vsb[:, kc * 520 : (kc + 1) * 520].rearrange(
                    "p (j d) -> p j d", d=65
                )[:, :, 0:64]
                src = ps[:].rearrange("p (j d) -> p j d", d=64)
                if with_bias:
                    bcv = bvbc_sb[:].rearrange("p (j d) -> p j d", d=64)
                    nc.vector.tensor_add(dst, src, bcv)
                else:
                    nc.vector.tensor_copy(dst, src)
                nc.gpsimd.tensor_copy(
                    vsb[:, kc * 520 + 64 : (kc + 1) * 520 : 65],
                    mc_sb[:, kc * 8 : (kc + 1) * 8],
                )

            # ---- projections: only K(dc0) + Q-qb0(dc0) up front (slot 0/1
            # attention needs just those); the rest interleave with
            # attention steps so the PE never drains and ACT starts early.
            def k_unit(dc, half):
                proj_unit(
                    "k", wk_sb, KT[dc][:, half * 512 : (half + 1) * 512],
                    dc, half, bk_sb if with_bias else None, nc.scalar,
                )

            def q_unit(dc, half):
                proj_unit(
                    "q", wq_sb, QT[dc][:, half * 512 : (half + 1) * 512],
                    dc, half, bq_sb if with_bias else None,
                    nc.scalar if half == 0 else nc.vector,
                )

            for dc in range(4):
                k_unit(dc, 0)
                k_unit(dc, 1)
            for dc in range(4):
                q_unit(dc, 0)
            pending_kq = []
            pending_v = [lambda kc=kc: v_unit(kc) for kc in range(8)]
            pending_q1 = [lambda dc=dc: q_unit(dc, 1) for dc in range(4)]
            v_next = [0]

            def need_v(kc):
                while v_next[0] <= kc and pending_v:
                    pending_v.pop(0)()
                    v_next[0] += 1

            mulc = [0]
            copc = [0]
            oqd_tiles = {}

            def attn_front(j, qb, kc):
                lo, hi = SPANS[(j, qb, kc)]
                w = hi - lo
                dc, rb = j // 2, (j % 2) * 64
                ps = psS.tile([128, 512], f32, tag="ps", name=f"ps{j}_{qb}_{kc}")
                nc.tensor.matmul(
                    ps[:, 0:w],
                    lhsT=KT[dc][rb : rb + 64, kc * 128 : (kc + 1) * 128],
                    rhs=QT[dc][rb : rb + 64, qb * 512 + lo : qb * 512 + hi],
                    start=True,
                    stop=True,
                )
                et = etp.tile([128, 512], bf16, tag="et", name=f"et{j}_{qb}_{kc}")
                nc.scalar.activation(et[:, 0:w], ps[:, 0:w], Exp)
                pt = ptp.tile([128, 512], bf16, tag="pt", name=f"pt{j}_{qb}_{kc}")
                off = kc * 128 - qb * 512
                for c0 in (0, 256):
                    a, bnd = max(lo, c0), min(hi, c0 + 256)
                    if a >= bnd:
                        continue
                    base = EBBASE[(j, off - c0)] * 256
                    mulc[0] += 1
                    eng = nc.gpsimd if mulc[0] % 6 == 0 else nc.vector
                    eng.tensor_mul(
                        pt[:, a - lo : bnd - lo],
                        et[:, a - lo : bnd - lo],
                        eb_sb[:, base + a - c0 : base + bnd - c0],
                    )
                return pt

            def pv_cluster(j, qb, pts):
                """PV matmuls for one (slot, qb), grouped by query chunk so
                each psum accumulation group opens and closes before the
                next starts (hw corrupts an open group when another group
                starts in the same bank). Returns a list of closures."""
                po = psO.tile([128, 512], f32, tag="po", name=f"po{j}_{qb}")
                ops = []

                def pv(qc, kc):
                    need_v(kc)  # vsb writer must be emitted before this read
                    lo, hi = SPANS[(j, qb, kc)]
                    kcs = COVER[(j, qb, qc)]
                    nc.tensor.matmul(
                        po[:, qc * 65 : (qc + 1) * 65],
                        lhsT=pts[kc][:, qc * 128 - lo : qc * 128 - lo + 128],
                        rhs=vsb[:, kc * 520 + j * 65 : kc * 520 + (j + 1) * 65],
                        start=(kc == kcs[0]),
                        stop=(kc == kcs[-1]),
                        skip_group_check=True,
                    )

                for qc in range(4):
                    if (j, qb, qc) not in COVER:
                        continue
                    for kc in COVER[(j, qb, qc)]:
                        ops.append(lambda qc=qc, kc=kc: pv(qc, kc))
                ops.append(lambda: norm(j, qb, po))
                return ops

            def norm(j, qb, po):
                den = rcp.tile([128, 4], f32, tag="rc", name=f"dn{j}{qb}")
                nc.vector.tensor_copy(den[:], po[:, 64:260:65])
                rc = rcp.tile([128, 4], f32, tag="rc", name=f"rc{j}{qb}")
                nc.vector.reciprocal(rc[:], den[:])
                if qb not in oqd_tiles:
                    oqd_tiles[qb] = [
                        oqdp.tile([128, 512], bf16, tag="oqd", name=f"oq{qb}_{qc}")
                        for qc in range(4)
                    ]
                for qc in range(4):
                    nc.vector.tensor_scalar_mul(
                        out=oqd_tiles[qb][qc][:, j * 64 : (j + 1) * 64],
                        in0=po[:, qc * 65 : qc * 65 + 64],
                        scalar1=rc[:, qc : qc + 1],
                    )

            def transpose_dc(qb, qc, dc, s8):
                pst = psW.tile([128, 128], bf16, tag="pw", name=f"tr{s8}{dc}")
                nc.tensor.transpose(
                    pst[:], oqd_tiles[qb][qc][:, dc * 128 : (dc + 1) * 128],
                    ident[:],
                )
                o8 = o8p.tile([128, 128], bf16, tag="o8", name=f"o8{s8}{dc}")
                copc[0] += 1
                eng = nc.vector if copc[0] % 2 == 0 else nc.scalar
                copy_to(eng, o8[:], pst[:])
                return o8

            def wo_unit(qb, qc):
                s8 = qb * 4 + qc
                o8t = [transpose_dc(qb, qc, dc, s8) for dc in range(4)]
                ob = obp.tile([128, 1024], bf16, tag="ob", name=f"ob{s8}")
                for half in range(2):
                    pw = psW.tile([128, 512], f32, tag="pw", name=f"wo{s8}{half}")
                    for dc in range(4):
                        nc.tensor.matmul(
                            pw[:],
                            lhsT=o8t[dc][:],
                            rhs=wo_sb[:, dc * 1024 + half * 512 : dc * 1024 + (half + 1) * 512],
                            start=(dc == 0),
                            stop=(dc == 3),
                        )
                    eng = nc.vector if half == 0 else nc.scalar
                    copy_to(eng, ob[:, half * 512 : (half + 1) * 512], pw[:])
                nc.sync.dma_start(
                    out=out[s8 * 128 : (s8 + 1) * 128, :], in_=ob[:]
                )

            obA = {}

            def woA_unit(qc):
                # qb1 Wo, dims half A (head-slots 0-3): runs as soon as their
                # norms land so only half the contraction remains in the tail.
                s8 = 4 + qc
                o8t = [transpose_dc(1, qc, dc, s8) for dc in range(2)]
                oa = obp.tile([128, 1024], bf16, tag="ob", name=f"oa{s8}")
                obA[qc] = oa
                for half in range(2):
                    pw = psW.tile([128, 512], f32, tag="pw", name=f"woA{s8}{half}")
                    for dc in range(2):
                        nc.tensor.matmul(
                            pw[:],
                            lhsT=o8t[dc][:],
                            rhs=wo_sb[:, dc * 1024 + half * 512 : dc * 1024 + (half + 1) * 512],
                            start=(dc == 0),
                            stop=(dc == 1),
                        )
                    eng = nc.vector if half == 0 else nc.scalar
                    copy_to(eng, oa[:, half * 512 : (half + 1) * 512], pw[:])

            def woB_unit(qc):
                s8 = 4 + qc
                o8t = [transpose_dc(1, qc, dc, s8) for dc in (2, 3)]
                ob = obp.tile([128, 1024], bf16, tag="ob", name=f"ob{s8}")
                for half in range(2):
                    pw = psW.tile([128, 512], f32, tag="pw", name=f"woB{s8}{half}")
                    for dcx, dc in enumerate((2, 3)):
                        nc.tensor.matmul(
                            pw[:],
                            lhsT=o8t[dcx][:],
                            rhs=wo_sb[:, dc * 1024 + half * 512 : dc * 1024 + (half + 1) * 512],
                            start=(dcx == 0),
                            stop=(dcx == 1),
                        )
                    nc.vector.tensor_add(
                        ob[:, half * 512 : (half + 1) * 512],
                        pw[:],
                        obA[qc][:, half * 512 : (half + 1) * 512],
                    )
                nc.sync.dma_start(
                    out=out[s8 * 128 : (s8 + 1) * 128, :], in_=ob[:]
                )

            # ---- software-pipelined attention ----
            # Slot (j, qb) fronts (QK -> exp -> EB-mult) interleave with the
            # previous slot's deferred PV cluster + norm, plus V/Q-qb1
            # projection units and qb0 Wo units, to keep every engine fed.
            # qb0 steep-first (small early clusters while V is still
            # projecting); qb1 steep-last (smallest possible tail cluster).
            slot_list = [(0, j) for j in range(NSLOT)]
            slot_list += [(1, j) for j in range(NSLOT)]
            back_q = []
            wo_q = []
            gi = [0]
            def bg_pop():
                if pending_v:
                    pending_v.pop(0)()
                    v_next[0] += 1
                elif pending_kq:
                    pending_kq.pop(0)()
                elif pending_q1:
                    pending_q1.pop(0)()

            for si, (qb, j) in enumerate(slot_list):
                if qb == 1 and (pending_kq or pending_q1):
                    while pending_kq:
                        pending_kq.pop(0)()
                    while pending_q1:
                        pending_q1.pop(0)()
                if si == 9:
                    wo_q.extend(lambda qc=qc: wo_unit(0, qc) for qc in range(4))
                if si == 13:
                    wo_q.extend(lambda qc=qc: woA_unit(qc) for qc in range(4))
                kcs = [
                    kc
                    for kc in range(4 if qb == 0 else 8)
                    if (j, qb, kc) in SPANS
                ]
                pts = {}
                for t, kc in enumerate(kcs):
                    pts[kc] = attn_front(j, qb, kc)
                    gi[0] += 1
                    if pending_v or pending_kq or pending_q1:
                        if gi[0] % 3 == 1 or len(pending_v) > 4:
                            bg_pop()
                    rem = len(kcs) - t
                    k = -(-len(back_q) // rem)  # empty back_q by slot end
                    for _ in range(k):
                        back_q.pop(0)()
                    if not back_q and wo_q and gi[0] % 2 == 0:
                        wo_q.pop(0)()
                while back_q:
                    back_q.pop(0)()
                back_q = pv_cluster(j, qb, pts)
            while back_q:
                back_q.pop(0)()
            while pending_kq or pending_v or pending_q1:
                bg_pop()
            while wo_q:
                wo_q.pop(0)()
            for qc in range(4):
                woB_unit(qc)

    if split:
        _split_sync_waits(nc)
    return nc


def _get_built(with_bias):
    key = bool(with_bias)
    if key not in _BUILT:
        _BUILT[key] = _build(key)
    return _BUILT[key]


def _prepare(inputs):
    query = np.asarray(inputs["query"], np.float32)
    key = np.asarray(inputs["key"], np.float32)
    value = np.asarray(inputs["value"], np.float32)
    kpm = np.asarray(inputs["key_padding_mask"], bool)
    Wq = np.asarray(inputs["Wq"], np.float32)
    bq = np.asarray(inputs["bq"], np.float32)
    Wk = np.asarray(inputs["Wk"], np.float32)
    bk = np.asarray(inputs["bk"], np.float32)
    Wv = np.asarray(inputs["Wv"], np.float32)
    bv = np.asarray(inputs["bv"], np.float32)
    Wo = np.asarray(inputs["Wo"], np.float32)

    scale = 1.0 / np.sqrt(np.float32(DK))
    with_bias = bool(np.any(bq) or np.any(bk) or np.any(bv))

    xq_b = [
        np.ascontiguousarray(query[b].T).astype(BF16) for b in range(B)
    ]
    xk_b = [np.ascontiguousarray(key[b].T).astype(BF16) for b in range(B)]
    xv_b = []
    for b in range(B):
        v = value[b].T.copy()  # [D, S]
        v[:, kpm[b]] = 0.0
        xv_b.append(np.ascontiguousarray(v).astype(BF16))
    # mcol[p, kc*8 + j] = live[kc*128 + p] for every slot j
    mcol_b = []
    for b in range(B):
        live = (~kpm[b]).astype(np.float32).reshape(8, 128)  # [kc, p]
        m = np.repeat(live.T[:, :, None], 8, axis=2)  # [p, kc, j]
        mcol_b.append(np.ascontiguousarray(m.reshape(128, 64)).astype(BF16))

    def pack_w(Ws):  # [512, 1024] -> [128, 4096] lhsT tiles (dc, ic)
        return np.ascontiguousarray(
            Ws.reshape(4, 128, 8, 128).transpose(3, 0, 2, 1).reshape(128, 4096)
        )

    def pack_wv(Ws):  # [512, 1024] -> rhs tiles [128, ic*512]
        return np.ascontiguousarray(
            Ws.T.reshape(8, 128, 512).transpose(1, 0, 2).reshape(128, 4096)
        )

    in_maps = []
    for c in range(N_CORES):
        b, half = c // 2, c % 2
        heads = [2 * j + half for j in range(NSLOT)]
        dsel = np.concatenate([np.arange(h * DK, (h + 1) * DK) for h in heads])
        wq_c = (Wq[dsel, :] * scale).astype(BF16)
        wk_c = Wk[dsel, :].astype(BF16)
        wv_c = Wv[dsel, :].astype(BF16)
        wo_c = np.ascontiguousarray(Wo[:, dsel].T).astype(np.float32)  # [512,1024]
        # wo rhs tiles: [128 dims(dc), 4 dc * (1024 outs)]
        wo_pack = np.ascontiguousarray(
            wo_c.reshape(4, 128, 1024).transpose(1, 0, 2).reshape(128, 4096)
        ).astype(BF16)

        eb = np.zeros((128, NEB * 256), np.float32)
        pp = np.arange(128)[:, None]
        cc = np.arange(256)[None, :]
        for j in range(NSLOT):
            sl = _slope(heads[j])
            for o in EBOFFS[j]:
                t = (pp - cc + o).astype(np.float32)
                tile = np.where(t <= 0, np.exp(sl * np.minimum(t, 0.0)), 0.0)
                eb[:, EBBASE[(j, o)] * 256 : (EBBASE[(j, o)] + 1) * 256] = tile
        im = {
            "xq": xq_b[b],
            "xk": xk_b[b],
            "xv": xv_b[b],
            "wq": pack_w(wq_c.astype(np.float32)).astype(BF16),
            "wk": pack_w(wk_c.astype(np.float32)).astype(BF16),
            "wv": pack_wv(wv_c.astype(np.float32)).astype(BF16),
            "wo": wo_pack,
            "ebp": eb.astype(BF16),
            "mcol": mcol_b[b],
        }
        if with_bias:
            im["bq"] = (bq[dsel] * scale).astype(np.float32).reshape(4, 128).T.copy()
            im["bk"] = bk[dsel].astype(np.float32).reshape(4, 128).T.copy()
            im["bv"] = bv[dsel].astype(np.float32).reshape(1, 512)
        in_maps.append(im)
    return with_bias, in_maps


def _run(inputs, trace=False):
    from concourse.bass_utils import run_bass_kernel_spmd

    with_bias, in_maps = _prepare(inputs)
    nc = _get_built(with_bias)
    res = run_bass_kernel_spmd(nc, in_maps, list(range(N_CORES)), trace=trace)
    acc = np.zeros((B, S, D), np.float32)
    for c in range(N_CORES):
        acc[c // 2] += np.asarray(res.results[c]["out"], np.float32)
    acc += np.asarray(inputs["bo"], np.float32)[None, None, :]
    return acc, res


def kernel(**inputs):
    out, _ = _run(inputs)
    return out
